# revision 27
# baseline (speedup 1.0000x reference)
"""Multi-head attention (B=4, S=2048, D=1024, H=16) on 8 TRN2 NeuronCores.

Sharding: core = (batch b = core//2, head-group g = core%2). Each core runs
8 heads (512 channels) of one batch element end-to-end; the host sums the two
head-group partials per batch and adds the constant bias term (bo + Wo@bv).

Device layouts (matmul operands bf16, PSUM/bias fp32):
  xqt/xkt/xvt [1024, 2048]   input.T per batch (bf16)
  wqt/wkt/wvt [128, 8, 512]  W_slice.T as [d_par, d_chunk, c] (bf16)
  wot         [128, 4, 1024] WoT_slice as [c_par, c_chunk, dout] (bf16)
  bq/bk       [128, 4]       per-partition bias fp32 (pre-softmax biases only)
  tria        [128, 128]     16*(i<=k)  -- causal ramp, lhsT (bf16)
  trib        [128, 128]     -15*(i>q)  -- causal ramp rhs, diagonal block only
Output: out_p [2048, 1024] fp32 partial (no bias).

Pipeline: the attention phase is paced by ScalarE exp (~(2N+414)/1.2 ns per
key-chunk vs ~2N/1.2 of tensor work), so projections for token block tb+1 and
the output projection of block qb-1 are emitted interleaved with attention(qb)
at key-chunk granularity -- the spare tensor slots hide the exp overhead.
PSUM: scores 2x2 banks, attn-out 1x2 banks, proj/outproj 2x1 bank.

Attention per (q-block, head pair): scoresT = kT.T@qT with a 128-wide
triangle-ramp matmul adding -240*(k-q)+ on the diagonal chunk (saturates exp
to 0 = causal mask); columns left of the diagonal chunk are skipped (off =
128*j); exp on ACT (scale=1/8, bounded scores); attnT accumulated via [V|1]
stationary (row 64 = denominators); normalize via reciprocal_approx_fast
(SBUF-staged -- custom DVE ops can't read PSUM) + partition broadcast.
"""

from collections import deque
from contextlib import ExitStack
from itertools import chain

import numpy as np

import concourse.bacc as bacc
import concourse.bass as bass
import concourse.mybir as mybir
import concourse.tile as tile
from concourse.bass_utils import run_bass_kernel_spmd

B, S, D, H, DK = 4, 2048, 1024, 16, 64
HL, C = 8, 512  # heads / channels per core
NCORES = 8
TB = 512  # token block for projections
QB = 512  # query block for attention
NTB = S // TB  # 4
NKC = S // 128  # 16 key chunks
DCH = D // 128  # 8 d-chunks
F32 = mybir.dt.float32
BF16 = mybir.dt.bfloat16
AF = mybir.ActivationFunctionType


def _emit_body(nc, tc, t):
    with ExitStack() as ctx:
        singles = ctx.enter_context(tc.tile_pool(name="singles", bufs=1))
        xtp = ctx.enter_context(tc.tile_pool(name="xtp", bufs=24))
        probs = ctx.enter_context(tc.tile_pool(name="probs", bufs=7))
        small = ctx.enter_context(tc.tile_pool(name="small", bufs=4))
        osb = ctx.enter_context(tc.tile_pool(name="osb", bufs=4))
        spP = ctx.enter_context(tc.tile_pool(name="spP", bufs=2, space="PSUM"))
        avP = ctx.enter_context(tc.tile_pool(name="avP", bufs=1, space="PSUM"))
        pjP = ctx.enter_context(tc.tile_pool(name="pjP", bufs=2, space="PSUM"))

        xr = {
            "q": t["xqt"].rearrange("(a p) tok -> p a tok", p=128),
            "k": t["xkt"].rearrange("(a p) tok -> p a tok", p=128),
            "v": t["xvt"].rearrange("(a p) tok -> p a tok", p=128),
        }
        # warm the three DMA queues (ring init) with tiny transfers so the
        # critical first loads below don't pay the setup cost
        warm = singles.tile([3, 4], F32, name="warm")
        nc.gpsimd.dma_start(warm[0:1, :], t["bqd"][0:1, :])
        nc.scalar.dma_start(warm[1:2, :], t["bqd"][0:1, :])
        nc.sync.dma_start(warm[2:3, :], t["bqd"][0:1, :])

        # the very first x tile rides the otherwise-idle gpsimd queue so the
        # first matmul only waits for it + the first wq chunk (~8.5us)
        xt_q0 = xtp.tile([128, TB], BF16, tag="xt", name="x_q_0_0")
        nc.gpsimd.dma_start(xt_q0[:], xr["q"][:, 0, 0:TB])

        # --- constants (issued from idle engine queues so the sync queue
        # serves the critical first x-tile loads immediately) ---
        tria_s = singles.tile([128, 128], BF16)
        nc.gpsimd.dma_start(tria_s[:], t["tria"][:])
        trib_s = singles.tile([128, 128], BF16)
        nc.gpsimd.dma_start(trib_s[:], t["trib"][:])
        bq_s = singles.tile([128, 4], F32)
        nc.gpsimd.dma_start(bq_s[:], t["bqd"][:])
        bk_s = singles.tile([128, 4], F32)
        nc.gpsimd.dma_start(bk_s[:], t["bkd"][:])

        # weights: wq loaded per-dc chunk (on the scalar queue) so the first
        # matmul can start as soon as chunk 0 + the first x tile land
        w_t = {}
        for which in ("q", "k", "v"):
            w_t[which] = singles.tile([128, DCH, C], BF16, name=f"w_{which}")
        for dc in range(DCH):
            nc.scalar.dma_start(w_t["q"][:, dc, :], t["wqt"][:, dc, :])
        for which, dram in (("k", "wkt"), ("v", "wvt")):
            for dc in range(DCH):
                nc.gpsimd.dma_start(w_t[which][:, dc, :], t[dram][:, dc, :])
        wv_s = w_t["v"]
        wo_s = singles.tile([128, 4, D], BF16, name="w_o")

        # --- persistent activations ---
        qT = {}  # (co, tb) -> [128, 512] c-partition, tokens free
        kT = {}
        for co in range(4):
            for tb in range(NTB):
                qT[co, tb] = singles.tile([128, TB], BF16, name=f"qT_{co}_{tb}")
                kT[co, tb] = singles.tile([128, TB], BF16, name=f"kT_{co}_{tb}")
        vS = {}  # kc -> [128 keys, 8 heads, 65] (channel 64 = ones)
        for kc in range(NKC):
            vS[kc] = singles.tile([128, HL, 65], BF16, name=f"v_{kc}")
            nc.gpsimd.memset(vS[kc][:, :, 64:65], 1.0)
        aT = {}  # (co, qb) -> [128, 512]
        for co in range(4):
            for qb in range(NTB):
                aT[co, qb] = singles.tile([128, QB], BF16, name=f"aT_{co}_{qb}")

        def copyback(dest, ps, b_s, co):
            # psum + per-partition bias -> bf16 SBUF, on DVE (keeps ACT free
            # for the exp stream)
            if b_s is None:
                nc.vector.tensor_copy(dest[:], ps[:])
            else:
                nc.vector.tensor_scalar_add(dest[:], ps[:], b_s[:, co : co + 1])

        def gen_proj_qk0(which):
            # token block 0, before attention exists: wide 2-bank tiles
            tb = 0
            w_s = w_t[which]
            b_s = bq_s if which == "q" else bk_s
            dest = qT if which == "q" else kT
            pss = [
                spP.tile([128, 2, QB], F32, tag="spb", name=f"psA_{which}_{tb}_{cop}")
                for cop in range(2)
            ]
            for dc in range(DCH):
                if which == "q" and dc == 0:
                    xt = xt_q0
                else:
                    xt = xtp.tile([128, TB], BF16, tag="xt", name=f"x_{which}_{tb}_{dc}")
                    nc.sync.dma_start(xt[:], xr[which][:, dc, tb * TB : (tb + 1) * TB])
                for co in range(4):
                    nc.tensor.matmul(
                        pss[co // 2][:, co % 2, :],
                        w_s[:, dc, co * 128 : (co + 1) * 128],
                        xt[:],
                        start=(dc == 0),
                        stop=(dc == DCH - 1),
                    )
                yield
            for co in range(4):
                nc.scalar.activation(
                    dest[co, tb][:],
                    pss[co // 2][:, co % 2, :],
                    AF.Identity,
                    bias=b_s[:, co : co + 1],
                )
            yield

        def gen_proj_v0():
            # token block 0 v: wide spb tiles (free after q(0)), single pass
            tb = 0
            pss = [
                spP.tile([128, 2, QB], F32, tag="spb", name=f"psV_{tb}_{kp}")
                for kp in range(2)
            ]
            for dc in range(DCH):
                xt = xtp.tile([128, TB], BF16, tag="xt", name=f"x_v_{tb}_{dc}")
                nc.sync.dma_start(xt[:], xr["v"][:, dc, tb * TB : (tb + 1) * TB])
                for kl in range(4):
                    nc.tensor.matmul(
                        pss[kl // 2][:, kl % 2, :],
                        xt[:, kl * 128 : (kl + 1) * 128],
                        wv_s[:, dc, :],
                        start=(dc == 0),
                        stop=(dc == DCH - 1),
                    )
                yield
            for kl in range(4):
                eng = nc.vector if kl % 2 == 0 else nc.scalar
                eng.tensor_copy(
                    vS[tb * 4 + kl][:, :, 0:64],
                    pss[kl // 2][:, kl % 2, :].rearrange("p (h e) -> p h e", h=HL),
                ) if kl % 2 == 0 else nc.scalar.copy(
                    vS[tb * 4 + kl][:, :, 0:64],
                    pss[kl // 2][:, kl % 2, :].rearrange("p (h e) -> p h e", h=HL),
                )
            yield

        def gen_proj_k0():
            # token block 0 k: two co-pair passes on the av tag so it can
            # pipeline against q(0) (spb) and v(0) (pj) in phase A
            tb = 0
            xts = {}
            for half in range(2):
                ps = avP.tile([128, 2, QB], F32, tag="av", name=f"psK0_{half}")
                for dc in range(DCH):
                    if half == 0:
                        xt = xtp.tile([128, TB], BF16, tag="xt", name=f"x_k_{tb}_{dc}")
                        nc.sync.dma_start(xt[:], xr["k"][:, dc, tb * TB : (tb + 1) * TB])
                        xts[dc] = xt
                    for cop in range(2):
                        co = 2 * half + cop
                        nc.tensor.matmul(
                            ps[:, cop, :],
                            w_t["k"][:, dc, co * 128 : (co + 1) * 128],
                            xts[dc][:],
                            start=(dc == 0),
                            stop=(dc == DCH - 1),
                        )
                    yield
                for cop in range(2):
                    co = 2 * half + cop
                    nc.scalar.activation(
                        kT[co, tb][:],
                        ps[:, cop, :],
                        AF.Identity,
                        bias=bk_s[:, co : co + 1],
                    )
                yield

        def gen_proj_qk(which, tb):
            # interleaved form: two co-pair passes over held x tiles,
            # 1-bank psum accumulators from the pj ring
            w_s = w_t[which]
            b_s = bq_s if which == "q" else bk_s
            dest = qT if which == "q" else kT
            xts = {}
            for half in range(2):
                pss = [
                    pjP.tile([128, QB], F32, tag="pj", name=f"psA_{which}_{tb}_{half}_{p}")
                    for p in range(2)
                ]
                for dc in range(DCH):
                    if half == 0:
                        xt = xtp.tile([128, TB], BF16, tag="xt", name=f"x_{which}_{tb}_{dc}")
                        nc.sync.dma_start(xt[:], xr[which][:, dc, tb * TB : (tb + 1) * TB])
                        xts[dc] = xt
                    for cop in range(2):
                        co = 2 * half + cop
                        nc.tensor.matmul(
                            pss[cop][:, :],
                            w_s[:, dc, co * 128 : (co + 1) * 128],
                            xts[dc][:],
                            start=(dc == 0),
                            stop=(dc == DCH - 1),
                        )
                    yield
                for cop in range(2):
                    co = 2 * half + cop
                    copyback(dest[co, tb], pss[cop], b_s, co)
                yield

        def gen_proj_v(tb):
            xts = {}
            for half in range(2):
                pss = [
                    pjP.tile([128, QB], F32, tag="pj", name=f"psV_{tb}_{half}_{p}")
                    for p in range(2)
                ]
                for dc in range(DCH):
                    if half == 0:
                        xt = xtp.tile([128, TB], BF16, tag="xt", name=f"x_v_{tb}_{dc}")
                        nc.sync.dma_start(xt[:], xr["v"][:, dc, tb * TB : (tb + 1) * TB])
                        xts[dc] = xt
                    for klp in range(2):
                        kl = 2 * half + klp
                        nc.tensor.matmul(
                            pss[klp][:, :],
                            xts[dc][:, kl * 128 : (kl + 1) * 128],
                            wv_s[:, dc, :],
                            start=(dc == 0),
                            stop=(dc == DCH - 1),
                        )
                    yield
                for klp in range(2):
                    kl = 2 * half + klp
                    nc.vector.tensor_copy(
                        vS[tb * 4 + kl][:, :, 0:64],
                        pss[klp].rearrange("p (h e) -> p h e", h=HL),
                    )
                yield

        def gen_attention(qb):
            n_kc = (qb + 1) * 4
            for hp in range(4):  # heads h0=2hp (par 0-63), h1 (par 64-127)
                co = hp
                av = avP.tile([128, 2, QB], F32, tag="av", name=f"av_{qb}_{hp}")

                def attn_v(kc, pt, off):
                    for hi in range(2):
                        nc.tensor.matmul(
                            av[0:65, hi, off:],
                            vS[kc][:, 2 * hp + hi, :],
                            pt[:, hi, off:],
                            start=(kc == 0),
                            stop=(kc == n_kc - 1),
                        )

                pend = deque()  # (kc, pt, off) whose exp may still be in flight
                for kc in range(n_kc):
                    j = kc - qb * 4
                    # columns < 128*j of a diagonal chunk are fully masked:
                    # skip them in scores/tri/exp/attnV entirely.
                    off = 128 * j if j >= 1 else 0
                    sp = spP.tile([128, 2, QB], F32, tag="spb", name=f"sp_{qb}_{hp}_{kc}")
                    for hi in range(2):
                        po = hi * 64
                        nc.tensor.matmul(
                            sp[:, hi, off:],
                            kT[co, kc // 4][po : po + 64, (kc % 4) * 128 : (kc % 4 + 1) * 128],
                            qT[co, qb][po : po + 64, off:],
                            start=True,
                            stop=(j < 0),
                        )
                    if j >= 0:
                        # ramp only needed on the 128-wide diagonal block
                        for hi in range(2):
                            nc.tensor.matmul(
                                sp[:, hi, 128 * j : 128 * (j + 1)],
                                tria_s[:],
                                trib_s[:],
                                start=False,
                                stop=True,
                            )
                    pt = probs.tile([128, 2, QB], BF16, tag="pt", name=f"pt_{qb}_{hp}_{kc}")
                    nc.scalar.activation(
                        pt[:, :, off:], sp[:, :, off:], AF.Exp, scale=0.125
                    )
                    pend.append((kc, pt, off))
                    if len(pend) > 5:
                        attn_v(*pend.popleft())
                    yield
                while pend:
                    attn_v(*pend.popleft())
                # normalize: row 64 of av = sum(exp). Custom DVE ops are
                # SBUF-only on hardware, so stage the denominators in SBUF.
                rec = small.tile([128, 2, QB], F32, tag="rec", name=f"rec_{qb}_{hp}")
                bcs = {}
                for hi in range(2):
                    den = small.tile([128, QB], F32, tag=f"den{hi}", name=f"den_{qb}_{hp}_{hi}")
                    nc.vector.tensor_copy(den[0:1, :], av[64:65, hi, :])
                    nc.vector.reciprocal_approx_fast(rec[0:1, hi, :], den[0:1, :])
                    bcs[hi] = small.tile([128, QB], F32, tag=f"bcs{hi}", name=f"bcs_{qb}_{hp}_{hi}")
                    nc.gpsimd.partition_broadcast(bcs[hi][:, :], rec[0:1, hi, :])
                for hi in range(2):
                    po = hi * 64
                    nc.vector.tensor_mul(
                        aT[co, qb][po : po + 64, :],
                        av[0:64, hi, :],
                        bcs[hi][po : po + 64, :],
                    )
                yield

        def gen_outproj(qb, act_copies=False):
            for qc in range(4):
                for do2 in range(2):
                    ps = pjP.tile([128, QB], F32, tag="pj", name=f"psC_{qb}_{qc}_{do2}")
                    for co in range(4):
                        nc.tensor.matmul(
                            ps[:, :],
                            aT[co, qb][:, qc * 128 : (qc + 1) * 128],
                            wo_s[:, co, do2 * 512 : (do2 + 1) * 512],
                            start=(co == 0),
                            stop=(co == 3),
                        )
                    ob = osb.tile([128, QB], F32, tag="ob", name=f"ob_{qb}_{qc}_{do2}")
                    if act_copies and (qc * 2 + do2) % 2 == 1:
                        nc.scalar.copy(ob[:], ps[:])
                    else:
                        nc.vector.tensor_copy(ob[:], ps[:])
                    nc.sync.dma_start(
                        t["out_p"][
                            qb * QB + qc * 128 : qb * QB + (qc + 1) * 128,
                            do2 * 512 : (do2 + 1) * 512,
                        ],
                        ob[:],
                    )
                    yield

        def gen_outproj_wide(qb):
            # post-attention: the spb/av rings are free; wide 2-bank tiles
            # let co0-2 of more groups pre-run during the last normalize,
            # and copies alternate DVE/ACT (ACT is idle once exps are done)
            for qc in range(4):
                pool, tag = (spP, "spb") if qc % 2 == 0 else (avP, "av")
                ps = pool.tile([128, 2, QB], F32, tag=tag, name=f"psCW_{qb}_{qc}")
                for do2 in range(2):
                    for co in range(4):
                        nc.tensor.matmul(
                            ps[:, do2, :],
                            aT[co, qb][:, qc * 128 : (qc + 1) * 128],
                            wo_s[:, co, do2 * 512 : (do2 + 1) * 512],
                            start=(co == 0),
                            stop=(co == 3),
                        )
                ob = osb.tile([128, 2, QB], F32, tag="obw", name=f"obw_{qb}_{qc}")
                if qc % 2 == 0:
                    nc.vector.tensor_copy(ob[:], ps[:])
                else:
                    nc.scalar.copy(ob[:], ps[:])
                nc.sync.dma_start(
                    t["out_p"].rearrange("(a p) d -> p a d", p=128)[
                        :, qb * 4 + qc, :
                    ],
                    ob.rearrange("p a d -> p (a d)")[:],
                )
                yield

        def drain(gen):
            for _ in gen:
                pass

        def interleave(main_gen, fill_gen, n_main, n_fill):
            acc = 0.0
            step = n_fill / n_main if n_main else 0.0
            for _ in main_gen:
                acc += step
                while acc >= 1.0:
                    if next(fill_gen, None) is None:
                        acc = 0.0
                        break
                    acc -= 1.0
            drain(fill_gen)

        # --- phase A: token block 0 projections; q/k/v sit on different
        # psum tags (spb / av / pj) so the scheduler pipelines them ---
        drain(gen_proj_qk0("q"))
        drain(gen_proj_k0())
        drain(gen_proj_v0())
        # --- pipelined phases: attention(qb) soaks up exp latency while the
        # tensor queue advances through proj(qb+1) and outproj(qb-1) ---
        for qb in range(NTB):
            if qb == 1:
                nc.gpsimd.dma_start(wo_s[:], t["wot"][:])
            n_main = 4 * ((qb + 1) * 4 + 1)
            if qb < NTB - 1:
                fills = [
                    gen_proj_qk("q", qb + 1),
                    gen_proj_qk("k", qb + 1),
                    gen_proj_v(qb + 1),
                ]
                n_fill = 54
            else:
                # all deferred output projections: attention(3) has the
                # largest exp overhang and nothing else left to fill it
                fills = [gen_outproj(0), gen_outproj(1), gen_outproj(2)]
                n_fill = 24
            interleave(gen_attention(qb), chain(*fills), n_main, n_fill)
        drain(gen_outproj_wide(NTB - 1))


_PROG = None


def _program():
    global _PROG
    if _PROG is not None:
        return _PROG
    nc = bacc.Bacc()
    t = {}
    t["xqt"] = nc.dram_tensor("xqt", [D, S], BF16, kind="ExternalInput")
    t["xkt"] = nc.dram_tensor("xkt", [D, S], BF16, kind="ExternalInput")
    t["xvt"] = nc.dram_tensor("xvt", [D, S], BF16, kind="ExternalInput")
    t["wqt"] = nc.dram_tensor("wqt", [128, DCH, C], BF16, kind="ExternalInput")
    t["wkt"] = nc.dram_tensor("wkt", [128, DCH, C], BF16, kind="ExternalInput")
    t["wvt"] = nc.dram_tensor("wvt", [128, DCH, C], BF16, kind="ExternalInput")
    t["wot"] = nc.dram_tensor("wot", [128, 4, D], BF16, kind="ExternalInput")
    t["bqd"] = nc.dram_tensor("bqd", [128, 4], F32, kind="ExternalInput")
    t["bkd"] = nc.dram_tensor("bkd", [128, 4], F32, kind="ExternalInput")
    t["tria"] = nc.dram_tensor("tria", [128, 128], BF16, kind="ExternalInput")
    t["trib"] = nc.dram_tensor("trib", [128, 128], BF16, kind="ExternalInput")
    t["out_p"] = nc.dram_tensor("out_p", [S, D], F32, kind="ExternalOutput")
    with tile.TileContext(nc) as tc:
        _emit_body(nc, tc, t)
    nc.compile()
    _PROG = nc
    return nc


def _host_tri():
    import ml_dtypes

    i = np.arange(128)[:, None]
    tria = (16.0 * (i <= np.arange(128)[None, :])).astype(ml_dtypes.bfloat16)
    trib = (-15.0 * (i > np.arange(128)[None, :])).astype(ml_dtypes.bfloat16)
    return tria, trib


def prepare_in_maps(Q, K, V, mask, Wq, bq, Wk, bk, Wv, bv, Wo, bo):
    import ml_dtypes

    BF = ml_dtypes.bfloat16
    tria, trib = _host_tri()

    def wslice(W, g):  # [128, 8, 512] lhsT layout of W_slice.T
        Wg = W[g * C : (g + 1) * C, :]  # [512, 1024]
        return np.ascontiguousarray(
            Wg.T.reshape(DCH, 128, C).transpose(1, 0, 2)
        ).astype(BF)

    def woslice(Wo_, g):  # [128, 4, 1024]
        Wg = Wo_[:, g * C : (g + 1) * C]  # [1024, 512]
        return np.ascontiguousarray(
            Wg.T.reshape(4, 128, D).transpose(1, 0, 2)
        ).astype(BF)

    def bslice(b, g):  # [128, 4]
        return np.ascontiguousarray(b[g * C : (g + 1) * C].reshape(4, 128).T).astype(
            np.float32
        )

    in_maps = []
    for core in range(NCORES):
        b, g = core // 2, core % 2
        in_maps.append(
            {
                "xqt": np.ascontiguousarray(np.asarray(Q)[b].T).astype(BF),
                "xkt": np.ascontiguousarray(np.asarray(K)[b].T).astype(BF),
                "xvt": np.ascontiguousarray(np.asarray(V)[b].T).astype(BF),
                "wqt": wslice(np.asarray(Wq), g),
                "wkt": wslice(np.asarray(Wk), g),
                "wvt": wslice(np.asarray(Wv), g),
                "wot": woslice(np.asarray(Wo), g),
                "bqd": bslice(np.asarray(bq), g),
                "bkd": bslice(np.asarray(bk), g),
                "tria": tria,
                "trib": trib,
            }
        )

    return in_maps


def gather_output(results, Wo, bv, bo):
    parts = [r["out_p"] for r in results]
    const = (np.asarray(Wo) @ np.asarray(bv) + np.asarray(bo)).astype(np.float32)
    return np.stack(
        [parts[2 * b] + parts[2 * b + 1] + const for b in range(B)]
    ).astype(np.float32)


def kernel(Q, K, V, mask, Wq, bq, Wk, bk, Wv, bv, Wo, bo):
    nc = _program()
    in_maps = prepare_in_maps(Q, K, V, mask, Wq, bq, Wk, bk, Wv, bv, Wo, bo)
    res = run_bass_kernel_spmd(nc, in_maps, list(range(NCORES)))
    return gather_output(res.results, Wo, bv, bo)


# revision 29
# speedup vs baseline: 1.0178x; 1.0178x over previous
"""Multi-head attention (B=4, S=2048, D=1024, H=16) on 8 TRN2 NeuronCores.

Sharding: core = (batch b = core//2, head-group g = core%2). Each core runs
8 heads (512 channels) of one batch element end-to-end; the host sums the two
head-group partials per batch and adds the constant bias term (bo + Wo@bv).

Device layouts (matmul operands bf16, PSUM/bias fp32):
  xqt/xkt/xvt [1024, 2048]   input.T per batch (bf16)
  wqt/wkt/wvt [128, 8, 512]  W_slice.T as [d_par, d_chunk, c] (bf16)
  wot         [128, 4, 1024] WoT_slice as [c_par, c_chunk, dout] (bf16)
  bq/bk       [128, 4]       per-partition bias fp32 (pre-softmax biases only)
  tria        [128, 128]     16*(i<=k)  -- causal ramp, lhsT (bf16)
  trib        [128, 128]     -15*(i>q)  -- causal ramp rhs, diagonal block only
Output: out_p [2048, 1024] fp32 partial (no bias).

Pipeline: the attention phase is paced by ScalarE exp (~(2N+414)/1.2 ns per
key-chunk vs ~2N/1.2 of tensor work), so projections for token block tb+1 and
the output projection of block qb-1 are emitted interleaved with attention(qb)
at key-chunk granularity -- the spare tensor slots hide the exp overhead.
PSUM: scores 2x2 banks, attn-out 1x2 banks, proj/outproj 2x1 bank.

Attention per (q-block, head pair): scoresT = kT.T@qT with a 128-wide
triangle-ramp matmul adding -240*(k-q)+ on the diagonal chunk (saturates exp
to 0 = causal mask); columns left of the diagonal chunk are skipped (off =
128*j); exp on ACT (scale=1/8, bounded scores); attnT accumulated via [V|1]
stationary (row 64 = denominators); normalize via reciprocal_approx_fast
(SBUF-staged -- custom DVE ops can't read PSUM) + partition broadcast.
"""

from collections import deque
from contextlib import ExitStack
from itertools import chain

import numpy as np

import concourse.bacc as bacc
import concourse.bass as bass
import concourse.mybir as mybir
import concourse.tile as tile
from concourse.bass_utils import run_bass_kernel_spmd

B, S, D, H, DK = 4, 2048, 1024, 16, 64
HL, C = 8, 512  # heads / channels per core
NCORES = 8
TB = 512  # token block for projections
QB = 512  # query block for attention
NTB = S // TB  # 4
NKC = S // 128  # 16 key chunks
DCH = D // 128  # 8 d-chunks
F32 = mybir.dt.float32
BF16 = mybir.dt.bfloat16
AF = mybir.ActivationFunctionType


def _emit_body(nc, tc, t):
    with ExitStack() as ctx:
        singles = ctx.enter_context(tc.tile_pool(name="singles", bufs=1))
        xtp = ctx.enter_context(tc.tile_pool(name="xtp", bufs=24))
        probs = ctx.enter_context(tc.tile_pool(name="probs", bufs=7))
        small = ctx.enter_context(tc.tile_pool(name="small", bufs=4))
        osb = ctx.enter_context(tc.tile_pool(name="osb", bufs=4))
        spP = ctx.enter_context(tc.tile_pool(name="spP", bufs=2, space="PSUM"))
        avP = ctx.enter_context(tc.tile_pool(name="avP", bufs=1, space="PSUM"))
        pjP = ctx.enter_context(tc.tile_pool(name="pjP", bufs=2, space="PSUM"))

        xr = {
            "q": t["xqt"].rearrange("(a p) tok -> p a tok", p=128),
            "k": t["xkt"].rearrange("(a p) tok -> p a tok", p=128),
            "v": t["xvt"].rearrange("(a p) tok -> p a tok", p=128),
        }
        # the very first x tile rides the otherwise-idle gpsimd queue so the
        # first matmul only waits for it + the first wq chunk (~8.5us)
        xt_q0 = xtp.tile([128, TB], BF16, tag="xt", name="x_q_0_0")
        nc.gpsimd.dma_start(xt_q0[:], xr["q"][:, 0, 0:TB])

        # --- constants (issued from idle engine queues so the sync queue
        # serves the critical first x-tile loads immediately) ---
        tria_s = singles.tile([128, 128], BF16)
        nc.gpsimd.dma_start(tria_s[:], t["tria"][:])
        trib_s = singles.tile([128, 128], BF16)
        nc.gpsimd.dma_start(trib_s[:], t["trib"][:])
        bq_s = singles.tile([128, 4], F32)
        nc.gpsimd.dma_start(bq_s[:], t["bqd"][:])
        bk_s = singles.tile([128, 4], F32)
        nc.gpsimd.dma_start(bk_s[:], t["bkd"][:])

        # weights: wq loaded per-dc chunk (on the scalar queue) so the first
        # matmul can start as soon as chunk 0 + the first x tile land
        w_t = {}
        for which in ("q", "k", "v"):
            w_t[which] = singles.tile([128, DCH, C], BF16, name=f"w_{which}")
        for dc in range(DCH):
            nc.scalar.dma_start(w_t["q"][:, dc, :], t["wqt"][:, dc, :])
        for which, dram in (("k", "wkt"), ("v", "wvt")):
            for dc in range(DCH):
                nc.gpsimd.dma_start(w_t[which][:, dc, :], t[dram][:, dc, :])
        wv_s = w_t["v"]
        wo_s = singles.tile([128, 4, D], BF16, name="w_o")

        # --- persistent activations ---
        qT = {}  # (co, tb) -> [128, 512] c-partition, tokens free
        kT = {}
        for co in range(4):
            for tb in range(NTB):
                qT[co, tb] = singles.tile([128, TB], BF16, name=f"qT_{co}_{tb}")
                kT[co, tb] = singles.tile([128, TB], BF16, name=f"kT_{co}_{tb}")
        vS = {}  # kc -> [128 keys, 8 heads, 65] (channel 64 = ones)
        for kc in range(NKC):
            vS[kc] = singles.tile([128, HL, 65], BF16, name=f"v_{kc}")
            nc.gpsimd.memset(vS[kc][:, :, 64:65], 1.0)
        aT = {}  # (co, qb) -> [128, 512]
        for co in range(4):
            for qb in range(NTB):
                aT[co, qb] = singles.tile([128, QB], BF16, name=f"aT_{co}_{qb}")

        def copyback(dest, ps, b_s, co):
            # psum + per-partition bias -> bf16 SBUF, on DVE (keeps ACT free
            # for the exp stream)
            if b_s is None:
                nc.vector.tensor_copy(dest[:], ps[:])
            else:
                nc.vector.tensor_scalar_add(dest[:], ps[:], b_s[:, co : co + 1])

        def gen_proj_qk0(which):
            # token block 0, before attention exists: wide 2-bank tiles
            tb = 0
            w_s = w_t[which]
            b_s = bq_s if which == "q" else bk_s
            dest = qT if which == "q" else kT
            pss = [
                spP.tile([128, 2, QB], F32, tag="spb", name=f"psA_{which}_{tb}_{cop}")
                for cop in range(2)
            ]
            for dc in range(DCH):
                if which == "q" and dc == 0:
                    xt = xt_q0
                else:
                    xt = xtp.tile([128, TB], BF16, tag="xt", name=f"x_{which}_{tb}_{dc}")
                    nc.sync.dma_start(xt[:], xr[which][:, dc, tb * TB : (tb + 1) * TB])
                for co in range(4):
                    nc.tensor.matmul(
                        pss[co // 2][:, co % 2, :],
                        w_s[:, dc, co * 128 : (co + 1) * 128],
                        xt[:],
                        start=(dc == 0),
                        stop=(dc == DCH - 1),
                    )
                yield
            for co in range(4):
                nc.scalar.activation(
                    dest[co, tb][:],
                    pss[co // 2][:, co % 2, :],
                    AF.Identity,
                    bias=b_s[:, co : co + 1],
                )
            yield

        def gen_proj_v0():
            # token block 0 v: wide spb tiles (free after q(0)), single pass
            tb = 0
            pss = [
                spP.tile([128, 2, QB], F32, tag="spb", name=f"psV_{tb}_{kp}")
                for kp in range(2)
            ]
            for dc in range(DCH):
                xt = xtp.tile([128, TB], BF16, tag="xt", name=f"x_v_{tb}_{dc}")
                nc.sync.dma_start(xt[:], xr["v"][:, dc, tb * TB : (tb + 1) * TB])
                for kl in range(4):
                    nc.tensor.matmul(
                        pss[kl // 2][:, kl % 2, :],
                        xt[:, kl * 128 : (kl + 1) * 128],
                        wv_s[:, dc, :],
                        start=(dc == 0),
                        stop=(dc == DCH - 1),
                    )
                yield
            for kl in range(4):
                nc.vector.tensor_copy(
                    vS[tb * 4 + kl][:, :, 0:64],
                    pss[kl // 2][:, kl % 2, :].rearrange("p (h e) -> p h e", h=HL),
                )
            yield

        def gen_proj_k0():
            # token block 0 k: two co-pair passes on the av tag so it can
            # pipeline against q(0) (spb) and v(0) (pj) in phase A
            tb = 0
            xts = {}
            for half in range(2):
                ps = avP.tile([128, 2, QB], F32, tag="av", name=f"psK0_{half}")
                for dc in range(DCH):
                    if half == 0:
                        xt = xtp.tile([128, TB], BF16, tag="xt", name=f"x_k_{tb}_{dc}")
                        nc.sync.dma_start(xt[:], xr["k"][:, dc, tb * TB : (tb + 1) * TB])
                        xts[dc] = xt
                    for cop in range(2):
                        co = 2 * half + cop
                        nc.tensor.matmul(
                            ps[:, cop, :],
                            w_t["k"][:, dc, co * 128 : (co + 1) * 128],
                            xts[dc][:],
                            start=(dc == 0),
                            stop=(dc == DCH - 1),
                        )
                    yield
                for cop in range(2):
                    co = 2 * half + cop
                    nc.scalar.activation(
                        kT[co, tb][:],
                        ps[:, cop, :],
                        AF.Identity,
                        bias=bk_s[:, co : co + 1],
                    )
                yield

        def gen_proj_qk(which, tb):
            # interleaved form: two co-pair passes over held x tiles,
            # 1-bank psum accumulators from the pj ring
            w_s = w_t[which]
            b_s = bq_s if which == "q" else bk_s
            dest = qT if which == "q" else kT
            xts = {}
            for half in range(2):
                pss = [
                    pjP.tile([128, QB], F32, tag="pj", name=f"psA_{which}_{tb}_{half}_{p}")
                    for p in range(2)
                ]
                for dc in range(DCH):
                    if half == 0:
                        xt = xtp.tile([128, TB], BF16, tag="xt", name=f"x_{which}_{tb}_{dc}")
                        nc.sync.dma_start(xt[:], xr[which][:, dc, tb * TB : (tb + 1) * TB])
                        xts[dc] = xt
                    for cop in range(2):
                        co = 2 * half + cop
                        nc.tensor.matmul(
                            pss[cop][:, :],
                            w_s[:, dc, co * 128 : (co + 1) * 128],
                            xts[dc][:],
                            start=(dc == 0),
                            stop=(dc == DCH - 1),
                        )
                    yield
                for cop in range(2):
                    co = 2 * half + cop
                    copyback(dest[co, tb], pss[cop], b_s, co)
                yield

        def gen_proj_v(tb):
            xts = {}
            for half in range(2):
                pss = [
                    pjP.tile([128, QB], F32, tag="pj", name=f"psV_{tb}_{half}_{p}")
                    for p in range(2)
                ]
                for dc in range(DCH):
                    if half == 0:
                        xt = xtp.tile([128, TB], BF16, tag="xt", name=f"x_v_{tb}_{dc}")
                        nc.sync.dma_start(xt[:], xr["v"][:, dc, tb * TB : (tb + 1) * TB])
                        xts[dc] = xt
                    for klp in range(2):
                        kl = 2 * half + klp
                        nc.tensor.matmul(
                            pss[klp][:, :],
                            xts[dc][:, kl * 128 : (kl + 1) * 128],
                            wv_s[:, dc, :],
                            start=(dc == 0),
                            stop=(dc == DCH - 1),
                        )
                    yield
                for klp in range(2):
                    kl = 2 * half + klp
                    nc.vector.tensor_copy(
                        vS[tb * 4 + kl][:, :, 0:64],
                        pss[klp].rearrange("p (h e) -> p h e", h=HL),
                    )
                yield

        def gen_attention(qb):
            n_kc = (qb + 1) * 4
            for hp in range(4):  # heads h0=2hp (par 0-63), h1 (par 64-127)
                co = hp
                av = avP.tile([128, 2, QB], F32, tag="av", name=f"av_{qb}_{hp}")

                def attn_v(kc, pt, off):
                    for hi in range(2):
                        nc.tensor.matmul(
                            av[0:65, hi, off:],
                            vS[kc][:, 2 * hp + hi, :],
                            pt[:, hi, off:],
                            start=(kc == 0),
                            stop=(kc == n_kc - 1),
                        )

                pend = deque()  # (kc, pt, off) whose exp may still be in flight
                for kc in range(n_kc):
                    j = kc - qb * 4
                    # columns < 128*j of a diagonal chunk are fully masked:
                    # skip them in scores/tri/exp/attnV entirely.
                    off = 128 * j if j >= 1 else 0
                    sp = spP.tile([128, 2, QB], F32, tag="spb", name=f"sp_{qb}_{hp}_{kc}")
                    for hi in range(2):
                        po = hi * 64
                        nc.tensor.matmul(
                            sp[:, hi, off:],
                            kT[co, kc // 4][po : po + 64, (kc % 4) * 128 : (kc % 4 + 1) * 128],
                            qT[co, qb][po : po + 64, off:],
                            start=True,
                            stop=(j < 0),
                        )
                    if j >= 0:
                        # ramp only needed on the 128-wide diagonal block
                        for hi in range(2):
                            nc.tensor.matmul(
                                sp[:, hi, 128 * j : 128 * (j + 1)],
                                tria_s[:],
                                trib_s[:],
                                start=False,
                                stop=True,
                            )
                    pt = probs.tile([128, 2, QB], BF16, tag="pt", name=f"pt_{qb}_{hp}_{kc}")
                    nc.scalar.activation(
                        pt[:, :, off:], sp[:, :, off:], AF.Exp, scale=0.125
                    )
                    pend.append((kc, pt, off))
                    if len(pend) > 5:
                        attn_v(*pend.popleft())
                    yield
                while pend:
                    attn_v(*pend.popleft())
                # normalize: row 64 of av = sum(exp). Custom DVE ops are
                # SBUF-only on hardware, so stage the denominators in SBUF.
                rec = small.tile([128, 2, QB], F32, tag="rec", name=f"rec_{qb}_{hp}")
                bcs = {}
                for hi in range(2):
                    den = small.tile([128, QB], F32, tag=f"den{hi}", name=f"den_{qb}_{hp}_{hi}")
                    nc.vector.tensor_copy(den[0:1, :], av[64:65, hi, :])
                    nc.vector.reciprocal_approx_fast(rec[0:1, hi, :], den[0:1, :])
                    bcs[hi] = small.tile([128, QB], F32, tag=f"bcs{hi}", name=f"bcs_{qb}_{hp}_{hi}")
                    nc.gpsimd.partition_broadcast(bcs[hi][:, :], rec[0:1, hi, :])
                for hi in range(2):
                    po = hi * 64
                    nc.vector.tensor_mul(
                        aT[co, qb][po : po + 64, :],
                        av[0:64, hi, :],
                        bcs[hi][po : po + 64, :],
                    )
                yield

        def gen_outproj(qb, act_copies=False):
            for qc in range(4):
                for do2 in range(2):
                    ps = pjP.tile([128, QB], F32, tag="pj", name=f"psC_{qb}_{qc}_{do2}")
                    for co in range(4):
                        nc.tensor.matmul(
                            ps[:, :],
                            aT[co, qb][:, qc * 128 : (qc + 1) * 128],
                            wo_s[:, co, do2 * 512 : (do2 + 1) * 512],
                            start=(co == 0),
                            stop=(co == 3),
                        )
                    ob = osb.tile([128, QB], F32, tag="ob", name=f"ob_{qb}_{qc}_{do2}")
                    if act_copies and (qc * 2 + do2) % 2 == 1:
                        nc.scalar.copy(ob[:], ps[:])
                    else:
                        nc.vector.tensor_copy(ob[:], ps[:])
                    nc.sync.dma_start(
                        t["out_p"][
                            qb * QB + qc * 128 : qb * QB + (qc + 1) * 128,
                            do2 * 512 : (do2 + 1) * 512,
                        ],
                        ob[:],
                    )
                    yield

        def gen_outproj_wide(qb):
            # post-attention: the spb/av rings are free; wide 2-bank tiles
            # let co0-2 of more groups pre-run during the last normalize,
            # and copies alternate DVE/ACT (ACT is idle once exps are done)
            for qc in range(4):
                if qc == 3:
                    # last group rides the two free narrow pj slots so it
                    # doesn't serialize behind qc1's copy+DMA on the av slot
                    pss = [
                        pjP.tile([128, QB], F32, tag="pj", name=f"psCW_{qb}_3_{do2}")
                        for do2 in range(2)
                    ]
                else:
                    pool, tag = (spP, "spb") if qc % 2 == 0 else (avP, "av")
                    ps = pool.tile([128, 2, QB], F32, tag=tag, name=f"psCW_{qb}_{qc}")
                    pss = [ps[:, 0, :], ps[:, 1, :]]
                for do2 in range(2):
                    for co in range(4):
                        nc.tensor.matmul(
                            pss[do2][:, :],
                            aT[co, qb][:, qc * 128 : (qc + 1) * 128],
                            wo_s[:, co, do2 * 512 : (do2 + 1) * 512],
                            start=(co == 0),
                            stop=(co == 3),
                        )
                ob = osb.tile([128, 2, QB], F32, tag="obw", name=f"obw_{qb}_{qc}")
                for do2 in range(2):
                    if qc % 2 == 0:
                        nc.vector.tensor_copy(ob[:, do2, :], pss[do2][:, :])
                    else:
                        nc.scalar.copy(ob[:, do2, :], pss[do2][:, :])
                nc.sync.dma_start(
                    t["out_p"].rearrange("(a p) d -> p a d", p=128)[
                        :, qb * 4 + qc, :
                    ],
                    ob.rearrange("p a d -> p (a d)")[:],
                )
                yield

        def drain(gen):
            for _ in gen:
                pass

        def interleave(main_gen, fill_gen, n_main, n_fill):
            acc = 0.0
            step = n_fill / n_main if n_main else 0.0
            for _ in main_gen:
                acc += step
                while acc >= 1.0:
                    if next(fill_gen, None) is None:
                        acc = 0.0
                        break
                    acc -= 1.0
            drain(fill_gen)

        # --- phase A: token block 0 projections; q/k/v sit on different
        # psum tags (spb / av / pj) so the scheduler pipelines them ---
        drain(gen_proj_qk0("q"))
        drain(gen_proj_k0())
        drain(gen_proj_v0())
        # --- pipelined phases: attention(qb) soaks up exp latency while the
        # tensor queue advances through proj(qb+1) and outproj(qb-1) ---
        for qb in range(NTB):
            if qb == 1:
                nc.gpsimd.dma_start(wo_s[:], t["wot"][:])
            n_main = 4 * ((qb + 1) * 4 + 1)
            if qb < NTB - 1:
                fills = [
                    gen_proj_qk("q", qb + 1),
                    gen_proj_qk("k", qb + 1),
                    gen_proj_v(qb + 1),
                ]
                n_fill = 54
            else:
                # all deferred output projections: attention(3) has the
                # largest exp overhang and nothing else left to fill it
                fills = [gen_outproj(0), gen_outproj(1), gen_outproj(2)]
                n_fill = 24
            interleave(gen_attention(qb), chain(*fills), n_main, n_fill)
        drain(gen_outproj_wide(NTB - 1))


_PROG = None


def _program():
    global _PROG
    if _PROG is not None:
        return _PROG
    nc = bacc.Bacc()
    t = {}
    t["xqt"] = nc.dram_tensor("xqt", [D, S], BF16, kind="ExternalInput")
    t["xkt"] = nc.dram_tensor("xkt", [D, S], BF16, kind="ExternalInput")
    t["xvt"] = nc.dram_tensor("xvt", [D, S], BF16, kind="ExternalInput")
    t["wqt"] = nc.dram_tensor("wqt", [128, DCH, C], BF16, kind="ExternalInput")
    t["wkt"] = nc.dram_tensor("wkt", [128, DCH, C], BF16, kind="ExternalInput")
    t["wvt"] = nc.dram_tensor("wvt", [128, DCH, C], BF16, kind="ExternalInput")
    t["wot"] = nc.dram_tensor("wot", [128, 4, D], BF16, kind="ExternalInput")
    t["bqd"] = nc.dram_tensor("bqd", [128, 4], F32, kind="ExternalInput")
    t["bkd"] = nc.dram_tensor("bkd", [128, 4], F32, kind="ExternalInput")
    t["tria"] = nc.dram_tensor("tria", [128, 128], BF16, kind="ExternalInput")
    t["trib"] = nc.dram_tensor("trib", [128, 128], BF16, kind="ExternalInput")
    t["out_p"] = nc.dram_tensor("out_p", [S, D], F32, kind="ExternalOutput")
    with tile.TileContext(nc) as tc:
        _emit_body(nc, tc, t)
    nc.compile()
    _PROG = nc
    return nc


def _host_tri():
    import ml_dtypes

    i = np.arange(128)[:, None]
    tria = (16.0 * (i <= np.arange(128)[None, :])).astype(ml_dtypes.bfloat16)
    trib = (-15.0 * (i > np.arange(128)[None, :])).astype(ml_dtypes.bfloat16)
    return tria, trib


def prepare_in_maps(Q, K, V, mask, Wq, bq, Wk, bk, Wv, bv, Wo, bo):
    import ml_dtypes

    BF = ml_dtypes.bfloat16
    tria, trib = _host_tri()

    def wslice(W, g):  # [128, 8, 512] lhsT layout of W_slice.T
        Wg = W[g * C : (g + 1) * C, :]  # [512, 1024]
        return np.ascontiguousarray(
            Wg.T.reshape(DCH, 128, C).transpose(1, 0, 2)
        ).astype(BF)

    def woslice(Wo_, g):  # [128, 4, 1024]
        Wg = Wo_[:, g * C : (g + 1) * C]  # [1024, 512]
        return np.ascontiguousarray(
            Wg.T.reshape(4, 128, D).transpose(1, 0, 2)
        ).astype(BF)

    def bslice(b, g):  # [128, 4]
        return np.ascontiguousarray(b[g * C : (g + 1) * C].reshape(4, 128).T).astype(
            np.float32
        )

    in_maps = []
    for core in range(NCORES):
        b, g = core // 2, core % 2
        in_maps.append(
            {
                "xqt": np.ascontiguousarray(np.asarray(Q)[b].T).astype(BF),
                "xkt": np.ascontiguousarray(np.asarray(K)[b].T).astype(BF),
                "xvt": np.ascontiguousarray(np.asarray(V)[b].T).astype(BF),
                "wqt": wslice(np.asarray(Wq), g),
                "wkt": wslice(np.asarray(Wk), g),
                "wvt": wslice(np.asarray(Wv), g),
                "wot": woslice(np.asarray(Wo), g),
                "bqd": bslice(np.asarray(bq), g),
                "bkd": bslice(np.asarray(bk), g),
                "tria": tria,
                "trib": trib,
            }
        )

    return in_maps


def gather_output(results, Wo, bv, bo):
    parts = [r["out_p"] for r in results]
    const = (np.asarray(Wo) @ np.asarray(bv) + np.asarray(bo)).astype(np.float32)
    return np.stack(
        [parts[2 * b] + parts[2 * b + 1] + const for b in range(B)]
    ).astype(np.float32)


def kernel(Q, K, V, mask, Wq, bq, Wk, bk, Wv, bv, Wo, bo):
    nc = _program()
    in_maps = prepare_in_maps(Q, K, V, mask, Wq, bq, Wk, bk, Wv, bv, Wo, bo)
    res = run_bass_kernel_spmd(nc, in_maps, list(range(NCORES)))
    return gather_output(res.results, Wo, bv, bo)
